# revision 9
# baseline (speedup 1.0000x reference)
"""Multi-head attention (B=4, S=2048, D=2048, H=16) on 8 trn2 NeuronCores.

Sharding: tensor-parallel over heads — 2 heads per core. Each core computes
its heads' Q/K/V projections, full attention for those heads, and a partial
output projection (its 256 rows of wo). The host sums the 8 partial outputs.

v5 (from the 940us fp32r baseline): the PE is the bottleneck, so PE work is
minimized and everything else is scheduled around keeping it fed:
  - all matmuls run in bf16 (same 1 cyc/row PE rate as fp32r, half the DMA
    and SBUF); fp8 DoubleRow blows the 2e-2 error budget (measured >=2.1e-2
    from any single fp8 tensor), so bf16 is the fastest usable dtype.
  - the softmax-denominator ones-matmul chain (8192 PE-cycles per head
    block, 14% of all PE work) is replaced by bf16 wide-adds on the DVE
    (2-byte fast mode) plus ONE 512-cycle bf16 matmul that partition-reduces
    and broadcasts the denominator in a single shot.
  - attention of batch b-1 is interleaved block-by-block between the
    projection spans of batch b: the projection phase needs almost no
    ACT/DVE help while attention saturates both (exp tiles, denominator
    adds, PSUM->SBUF casts), so mixing them keeps every engine under ~75%
    of the PE's pace instead of alternating 100%-idle / 130%-oversubscribed.
  - each block's denominator matmul + reciprocal + normalize are deferred
    past the next projection span so the PE never waits on the DVE chain.
  - the out-projection of query-span qs-1 drains one item per score/AV pair
    inside span qs's blocks (PSUM ring pressure limits lookahead to ~2
    items); its casts run mostly on DVE, 2 per span on ACT.
  - input DMAs ride the ACT hardware queue, weights/outputs ride the SP
    queue, so x spans and weight tensors stream in parallel at startup.
  - outputs ship as bf16 partials (half the writeback DMA), host sums fp32.
"""
import os
import sys

sys.path.insert(0, "/opt/trn_rl_repo")
import numpy as np
import ml_dtypes

B, S, D, H = 4, 2048, 2048, 16
HD = 128
NCORES = 8
HP = H // NCORES          # heads per core = 2
DC = HP * HD              # per-core slice of D = 256
TOK = B * S               # 8192
SCALE = HD ** -0.5
NDC = D // 128            # 16 contraction chunks for the projections
SPAN = 256                # token span per projection step
NSPAN = S // SPAN         # 8 spans per batch
QS = 512                  # query span in attention
NQS = S // QS             # 4
NKC = S // 128            # 16 key chunks
NKP = NKC // 2            # 8 key-chunk pairs (one exp tile each)

LAST_EXEC_NS = None
_BUILT = None


def _build():
    global _BUILT
    if _BUILT is not None:
        return _BUILT
    import concourse.tile as tile
    from concourse import bacc, mybir

    BF16 = mybir.dt.bfloat16
    F32 = mybir.dt.float32
    Exp = mybir.ActivationFunctionType.Exp
    Ident = mybir.ActivationFunctionType.Identity

    nc = bacc.Bacc("TRN2", target_bir_lowering=False, debug=False)
    xt = nc.dram_tensor("xt", [D, TOK], BF16, kind="ExternalInput")
    wq = nc.dram_tensor("wq", [D, DC], BF16, kind="ExternalInput")
    wk = nc.dram_tensor("wk", [D, DC], BF16, kind="ExternalInput")
    wv = nc.dram_tensor("wv", [D, DC], BF16, kind="ExternalInput")
    wo = nc.dram_tensor("wo", [DC, D], BF16, kind="ExternalInput")
    bq2 = nc.dram_tensor("bq2", [HD, HP], F32, kind="ExternalInput")
    bk2 = nc.dram_tensor("bk2", [HD, HP], F32, kind="ExternalInput")
    ones = nc.dram_tensor("ones", [128, 128], BF16, kind="ExternalInput")
    out = nc.dram_tensor("out", [TOK, D], BF16, kind="ExternalOutput")

    with tile.TileContext(nc) as tc:
        with tc.tile_pool(name="const", bufs=1) as cpool, \
             tc.tile_pool(name="xp", bufs=3) as xpool, \
             tc.tile_pool(name="bt", bufs=1) as bpool, \
             tc.tile_pool(name="at", bufs=3) as apool, \
             tc.tile_pool(name="ot", bufs=2) as opool, \
             tc.tile_pool(name="ps", bufs=1, space="PSUM") as ps:

            wq_sb = cpool.tile([128, NDC, DC], BF16)
            wk_sb = cpool.tile([128, NDC, DC], BF16)
            wv_sb = cpool.tile([128, NDC, DC], BF16)
            wo_sb = cpool.tile([128, HP, D], BF16)
            ones_sb = cpool.tile([128, 128], BF16)
            bq_sb = cpool.tile([HD, HP], F32)
            bk_sb = cpool.tile([HD, HP], F32)
            # weights stream on the SP queue; biases + x spans on the ACT
            # queue so the first projection isn't gated by one serial queue
            nc.sync.dma_start(out=wq_sb, in_=wq.rearrange("(c p) n -> p c n", p=128))
            nc.scalar.dma_start(out=bq_sb, in_=bq2[:, :])
            nc.scalar.dma_start(out=bk_sb, in_=bk2[:, :])

            xt_r = xt.rearrange("(c p) t -> p c t", p=128)

            proj_tiles = {}
            avt_tiles = {}
            pending = []       # deferred out-projection items
            fin_pending = []   # deferred block finalize (dn mm/recip/mul)
            osb_state = {}

            def emit_outproj_item(idx):
                b_i, tch, dsp = pending[idx]
                if dsp == 0:
                    osb_state[(b_i, tch)] = opool.tile(
                        [128, D], BF16, name="out_sb", tag="out_sb")
                out_sb = osb_state[(b_i, tch)]
                ops = ps.tile([128, 512], F32, name="ops", tag="pj", bufs=2)
                avt_b_i = avt_tiles[b_i % 2]
                for h in range(HP):
                    nc.tensor.matmul(
                        ops, avt_b_i[:, h, tch * 128:(tch + 1) * 128],
                        wo_sb[:, h, dsp * 512:(dsp + 1) * 512],
                        start=(h == 0), stop=(h == HP - 1))
                # most casts on DVE; 2 per span on ACT (Pool cannot access
                # PSUM on TRN2)
                if dsp == 3 and tch % 2 == 1:
                    nc.scalar.copy(out_sb[:, dsp * 512:(dsp + 1) * 512], ops)
                else:
                    nc.vector.tensor_copy(
                        out_sb[:, dsp * 512:(dsp + 1) * 512], ops)
                if dsp == D // 512 - 1:
                    nc.sync.dma_start(
                        out=out[b_i * S + tch * 128:
                                b_i * S + (tch + 1) * 128, :],
                        in_=out_sb)
                    del osb_state[(b_i, tch)]

            def drain(n):
                for _ in range(n):
                    if not pending:
                        return
                    emit_outproj_item(0)
                    pending.pop(0)

            def emit_finalize():
                # denominator partition-reduce + broadcast (one 512-cycle
                # bf16 matmul), reciprocal, and the flash-style normalize —
                # emitted one unit late so the PE never waits on the DVE
                # add-chain tail
                for b_i, qs, h, av_ps, dn_ps, dfold in fin_pending:
                    nc.tensor.matmul(dn_ps, ones_sb, dfold,
                                     start=True, stop=True)
                    recip = apool.tile([128, QS], F32, name="recip",
                                       tag="recip", bufs=2)
                    nc.vector.reciprocal_approx_fast(recip, dn_ps)
                    nc.vector.tensor_mul(
                        avt_tiles[b_i % 2][:, h, qs * QS:(qs + 1) * QS],
                        av_ps, recip)
                fin_pending.clear()

            def emit_att_block(b_i, qs, h):
                emit_finalize()
                qt_b, kt_b, v_b = proj_tiles[b_i % 2]
                if qs == 0 and h == 0:
                    avt_tiles[b_i % 2] = bpool.tile(
                        [128, HP, S], BF16, name="avt_b", tag="avt_b", bufs=2)
                avt_b = avt_tiles[b_i % 2]
                q_sl = qt_b[:, h, qs * QS:(qs + 1) * QS]
                av_ps = ps.tile([HD, QS], F32, name="av_ps", tag="acc",
                                bufs=2)
                dn_ps = ps.tile([128, QS], F32, name="dn_ps", tag="acc",
                                bufs=2)
                # running key-sum of the exp tiles (bf16 wide-adds on DVE)
                dacc = apool.tile([128, 2 * QS], BF16, name="dacc",
                                  tag="dacc", bufs=2)

                def emit_av(kp, p_prev):
                    for j in range(2):
                        kc = 2 * kp + j
                        nc.tensor.matmul(
                            av_ps, v_b[:, kc, h * HD:(h + 1) * HD],
                            p_prev[:, j * QS:(j + 1) * QS],
                            start=(kc == 0), stop=(kc == NKC - 1))
                    if kp == 0:
                        nc.vector.tensor_copy(dacc, p_prev)
                    else:
                        nc.vector.tensor_add(dacc, dacc, p_prev)

                p_prev = None
                for kp in range(NKP):
                    # two key-chunks share one psum tile and one exp; AV of
                    # pair kp-1 is emitted after the scores of pair kp so the
                    # PE never head-of-line blocks on the exp it needs; one
                    # out-projection item of the previous span drains per
                    # pair to give the exp pipeline PE work to hide behind
                    s_ps = ps.tile([128, 2 * QS], F32, name="s_ps", tag="s",
                                   bufs=2)
                    p_sb = apool.tile([128, 2 * QS], BF16, name="p_sb",
                                      tag="p", bufs=4)
                    for j in range(2):
                        kc = 2 * kp + j
                        nc.tensor.matmul(
                            s_ps[:, j * QS:(j + 1) * QS],
                            kt_b[:, h, kc * 128:(kc + 1) * 128], q_sl,
                            start=True, stop=True)
                    nc.scalar.activation(p_sb, s_ps, Exp, scale=SCALE)
                    if p_prev is not None:
                        emit_av(kp - 1, p_prev)
                    drain(1)
                    p_prev = p_sb
                emit_av(NKP - 1, p_prev)
                # fold the two QS halves; the partition-reduce matmul is in
                # the deferred finalize
                dfold = apool.tile([128, QS], BF16, name="dfold", tag="dfold",
                                   bufs=2)
                nc.vector.tensor_add(dfold, dacc[:, 0:QS], dacc[:, QS:2 * QS])
                fin_pending.append((b_i, qs, h, av_ps, dn_ps, dfold))
                if h == HP - 1:
                    for tloc in range(QS // 128):
                        tch = qs * (QS // 128) + tloc
                        for dsp in range(D // 512):
                            pending.append((b_i, tch, dsp))

            def emit_proj_span(b, sp):
                t0 = b * S + sp * SPAN
                qt_b, kt_b, v_b = proj_tiles[b % 2]
                xsp = xpool.tile([128, NDC, SPAN], BF16, name="xsp",
                                 tag="xsp")
                nc.scalar.dma_start(out=xsp, in_=xt_r[:, :, t0:t0 + SPAN])
                if b == 0 and sp == 0:
                    # wk/wv queue behind wq on the SP queue while the first
                    # x span streams on the ACT queue
                    nc.sync.dma_start(
                        out=wk_sb, in_=wk.rearrange("(c p) n -> p c n", p=128))
                    nc.sync.dma_start(
                        out=wv_sb, in_=wv.rearrange("(c p) n -> p c n", p=128))
                for h in range(HP):
                    # Q and K accumulate into halves of one PSUM bank
                    qkps = ps.tile([128, 2 * SPAN], F32, name="qkps",
                                   tag="pj", bufs=2)
                    for c in range(NDC):
                        nc.tensor.matmul(
                            qkps[:, 0:SPAN], wq_sb[:, c, h * HD:(h + 1) * HD],
                            xsp[:, c, :], start=(c == 0), stop=(c == NDC - 1))
                    for c in range(NDC):
                        nc.tensor.matmul(
                            qkps[:, SPAN:2 * SPAN],
                            wk_sb[:, c, h * HD:(h + 1) * HD],
                            xsp[:, c, :], start=(c == 0), stop=(c == NDC - 1))
                    nc.scalar.activation(
                        qt_b[:, h, sp * SPAN:(sp + 1) * SPAN],
                        qkps[:, 0:SPAN], Ident, bias=bq_sb[:, h:h + 1])
                    nc.scalar.activation(
                        kt_b[:, h, sp * SPAN:(sp + 1) * SPAN],
                        qkps[:, SPAN:2 * SPAN], Ident, bias=bk_sb[:, h:h + 1])
                # both V token-chunks accumulate into one PSUM bank
                vps = ps.tile([128, 2 * DC], F32, name="vps", tag="pj",
                              bufs=2)
                for tch in range(SPAN // 128):
                    for c in range(NDC):
                        nc.tensor.matmul(
                            vps[:, tch * DC:(tch + 1) * DC],
                            xsp[:, c, tch * 128:(tch + 1) * 128],
                            wv_sb[:, c, :], start=(c == 0), stop=(c == NDC - 1))
                for tch in range(SPAN // 128):
                    nc.scalar.copy(
                        v_b[:, sp * (SPAN // 128) + tch, :],
                        vps[:, tch * DC:(tch + 1) * DC])

            for b in range(B):
                proj_tiles[b % 2] = (
                    bpool.tile([128, HP, S], BF16, name="qt_b", tag="qt_b",
                               bufs=2),
                    bpool.tile([128, HP, S], BF16, name="kt_b", tag="kt_b",
                               bufs=2),
                    bpool.tile([128, NKC, DC], BF16, name="v_b", tag="v_b",
                               bufs=2),
                )
                for sp in range(NSPAN):
                    emit_proj_span(b, sp)
                    if b > 0:
                        # one attention block of the previous batch per
                        # projection span: 8 spans x (4 qs-spans x 2 heads)
                        emit_att_block(b - 1, sp // 2, sp % 2)
                if b == 0:
                    nc.sync.dma_start(
                        out=wo_sb, in_=wo.rearrange("(c p) n -> p c n", p=128))
                    nc.sync.dma_start(out=ones_sb, in_=ones[:, :])

            # last batch's attention has no projection phase left to hide in
            for qs in range(NQS):
                for h in range(HP):
                    emit_att_block(B - 1, qs, h)
            emit_finalize()
            drain(len(pending))
    nc.compile()
    _BUILT = nc
    return nc


def _install_trace_hooks():
    import types
    try:
        import antenv.axon_hooks  # noqa: F401
        return True
    except ImportError:
        pass
    try:
        from trn_agent_boot.trn_boot import _ntff_profile_via_ctypes
        hook = _ntff_profile_via_ctypes('/opt/axon/libaxon_pjrt.so')
        if hook is None:
            return False
        m = types.ModuleType('antenv.axon_hooks')
        m.get_axon_ntff_profile_hook = lambda: hook
        sys.modules['antenv.axon_hooks'] = m
        from concourse import bass_utils
        bass_utils.upload_artifacts = lambda tmpdir: "local://" + tmpdir
        return True
    except Exception:
        return False


def kernel(x, wq, bq, wk, bk, wv, bv, wo, bo):
    global LAST_EXEC_NS
    from concourse.bass_utils import run_bass_kernel_spmd

    BF16 = ml_dtypes.bfloat16

    x = np.asarray(x, dtype=np.float32)
    wq = np.asarray(wq, dtype=np.float32)
    bq = np.asarray(bq, dtype=np.float32)
    wk = np.asarray(wk, dtype=np.float32)
    bk = np.asarray(bk, dtype=np.float32)
    wv = np.asarray(wv, dtype=np.float32)
    bv = np.asarray(bv, dtype=np.float32)
    wo = np.asarray(wo, dtype=np.float32)
    bo = np.asarray(bo, dtype=np.float32)

    xt = np.ascontiguousarray(x.reshape(TOK, D).T).astype(BF16)
    ones = np.ones((128, 128), dtype=np.float32).astype(BF16)
    in_maps = []
    for i in range(NCORES):
        sl = slice(i * DC, (i + 1) * DC)
        in_maps.append({
            "xt": xt,
            "wq": wq[:, sl].astype(BF16),
            "wk": wk[:, sl].astype(BF16),
            "wv": wv[:, sl].astype(BF16),
            "wo": wo[sl, :].astype(BF16),
            "bq2": np.ascontiguousarray(bq[sl].reshape(HP, HD).T),
            "bk2": np.ascontiguousarray(bk[sl].reshape(HP, HD).T),
            "ones": ones,
        })

    trace = bool(os.environ.get("KERNEL_TRACE"))
    if trace:
        trace = _install_trace_hooks()

    nc = _build()
    res = run_bass_kernel_spmd(nc, in_maps, list(range(NCORES)), trace=trace)
    LAST_EXEC_NS = res.exec_time_ns

    total = np.zeros((TOK, D), dtype=np.float32)
    for r in res.results:
        total += r["out"].astype(np.float32)
    # V-bias folds into a constant row: softmax rows sum to 1, so
    # attention(V + 1*bv^T) = attention(V) + 1*bv^T, and (bv @ wo) adds to bo.
    total += bo + bv @ wo
    return total.reshape(B, S, D)
